# revision 1
# baseline (speedup 1.0000x reference)
"""Trainium2 Bass kernel for the two-branch (spatial/temporal) attention module.

Computation (full, fp32 reference):
    qkv = x @ Wqkv; q,k,v split -> heads [b,8,n,64]; half = n//2
    all 4096 queries attend to k_t (keys 2048:4096); softmax; out rows
    0:2048 read v rows 0:2048 (spatial), rows 2048:4096 read v rows
    2048:4096 (temporal); concat heads; out @ Wout + b_out.

Sharding (8 cores): core c handles batch c//4 and heads {2*(c%4), 2*(c%4)+1}.
Each core computes its 2 heads' q/k/v projections (tensor-parallel columns of
Wqkv), full attention for those heads, and a partial output projection using
its 128 rows of Wout. The host sums the 4 partial outputs per batch (the
"all-reduce") and adds b_out.

On-chip layout is fully transposed (feature dims on partitions) so no big
transposes are needed:
  xT [512,4096](bf16, host-cast) -> qT/kT_t/vT [128(2h x 64d), n] via
    lhsT=Wqkv tiles (k_s is never computed - the module doesn't use it)
  dots_T [j,i] via lhsT=kT-tile, rhs=qT   (K=64; the two heads' matmuls are
    emitted adjacently at row strips 0:64 / 64:128 so they pack concurrently)
  E = exp(SCALE*dots_T) on ScalarE (no max subtraction needed: |logits|<~3)
  out+sums via lhsT=[v_h|1] (v natural from small PE transposes), rhs=E
  normalization: sums row scatter-DMA'd p-major to [128,8] so the DVE
    reciprocal runs 128-lanes-wide, then a stride-0 DMA broadcast + multiply
  P^T [e,n] via lhsT=Wout-tile (f32r), rhs=A^T, as a short warm tail.

Perf notes (HW-measured): the PE must never idle >3.4us or the HAM clock
gate drops it to 1.2GHz (2x matmul cost, 4x for f32r) - hence the warm-up
spin during the DMA fill and the pure-matmul PE stream through attention.
reciprocal_approx_fast returns garbage on HW via this compile path; plain
reciprocal on [1,N] runs 6.4ns/elem single-lane - both are avoided.
"""

import sys

sys.path.insert(0, "/opt/trn_rl_repo")

import ml_dtypes
import numpy as np

import concourse.bass as bass
import concourse.mybir as mybir
import concourse.tile as tile
from concourse import bacc
from concourse.bass_utils import run_bass_kernel_spmd
from concourse.masks import make_identity

F32 = mybir.dt.float32
F32R = mybir.dt.float32r
BF16 = mybir.dt.bfloat16

N = 4096
HALF = 2048
DIM = 512
D = 64  # dim head
SCALE = DIM ** -0.5


def build_nc():
    nc = bacc.Bacc("TRN2", target_bir_lowering=False, debug=False)

    xT_d = nc.dram_tensor("xT", [DIM, N], BF16, kind="ExternalInput")
    wqkv_d = nc.dram_tensor("Wqkv", [DIM, 384], BF16, kind="ExternalInput")
    wout_d = nc.dram_tensor("Wout", [128, DIM], F32R, kind="ExternalInput")
    outT_d = nc.dram_tensor("outT", [DIM, N], F32, kind="ExternalOutput")

    AF = mybir.ActivationFunctionType

    with tile.TileContext(nc) as tc:
        with (
            tc.tile_pool(name="persist", bufs=1) as persist,
            tc.tile_pool(name="pm", bufs=2, space="PSUM") as pm,  # 2x2 banks
            tc.tile_pool(name="pa", bufs=2, space="PSUM") as pa,  # 2x2 banks
            tc.tile_pool(name="es", bufs=4) as es,
            tc.tile_pool(name="sm", bufs=4) as sm,
            tc.tile_pool(name="osb", bufs=6) as osb,
            tc.tile_pool(name="dr", bufs=4, space="DRAM") as dr,
        ):
            # per-1024-chunk q^T tiles; k^T only for keys 2048:4096; v^T full
            qTc = [persist.tile([128, 1024], BF16, tag=f"qT{i}", name=f"qT{i}") for i in range(4)]
            kTt = persist.tile([128, HALF], BF16, tag="kTt")
            vTc = [persist.tile([128, 1024], BF16, tag=f"vT{i}", name=f"vT{i}") for i in range(4)]
            # v natural + ones column: [j-part, jtile(32 over full n), head, 65]
            vp = persist.tile([128, 32, 2, 65], BF16, tag="vp")
            wq_s = persist.tile([128, 4, 384], BF16, tag="wq")
            wout_s = persist.tile([128, DIM], F32R, tag="wout")
            ident = persist.tile([128, 128], BF16, tag="ident")
            ATc = [persist.tile([128, 1024], F32R, tag=f"AT{i}", name=f"AT{i}") for i in range(4)]
            xt = [
                [persist.tile([128, HALF], BF16, tag=f"xt{i}_{nh}", name=f"xt{i}_{nh}") for nh in range(2)]
                for i in range(4)
            ]

            # ---------------- loads ------------------------------------------
            nc.sync.dma_start(
                out=wq_s[:, :, :],
                in_=wqkv_d[:, :].rearrange("(t p) c -> p t c", p=128),
            )
            for nh in (1, 0):
                for ct in range(4):
                    nc.sync.dma_start(
                        out=xt[ct][nh][:, :],
                        in_=xT_d[128 * ct : 128 * (ct + 1), 2048 * nh : 2048 * (nh + 1)],
                    )
            nc.sync.dma_start(out=wout_s[:, :], in_=wout_d[:, :])
            make_identity(nc, ident[:, :])
            nc.vector.memset(vp[:, :, :, 64:65], 1.0)

            # PE warm-up: ~5us of dummy matmuls while the x DMAs stream, so
            # the HAM clock gate reaches K=8/8 before the real projection
            # matmuls issue (it would otherwise run them at 1.2GHz).
            spin = persist.tile([128, 512], BF16, tag="spin")
            nc.vector.memset(spin[:, :], 1.0)
            def spin_mms(k):
                wps = pm.tile([128, 512], F32, tag="mm", name="wps")
                for _ in range(k):
                    nc.tensor.matmul(
                        out=wps[:, :], lhsT=spin[:, 0:128], rhs=spin[:, :],
                        start=True, stop=True,
                    )

            spin_mms(16)

            # ---------------- qkv^T projection -------------------------------
            # out[col, n] = Wqkv_c[:, col]^T @ x^T ; accumulate over 4 c-tiles
            def proj(dst, wcol0, n0):
                ps = pm.tile([128, 1024], F32, tag="mm", name="ps")
                nh, nb = n0 // HALF, n0 % HALF
                for hf in range(2):
                    for ct in range(4):
                        nc.tensor.matmul(
                            out=ps[:, 512 * hf : 512 * (hf + 1)],
                            lhsT=wq_s[:, ct, wcol0 : wcol0 + 128],
                            rhs=xt[ct][nh][:, nb + 512 * hf : nb + 512 * (hf + 1)],
                            start=(ct == 0),
                            stop=(ct == 3),
                        )
                nc.vector.tensor_copy(out=dst, in_=ps[:, :])

            def vtrans(jt):
                tp = pm.tile([128, 128], BF16, tag="mm", name="tp")
                nc.tensor.transpose(
                    tp[:, :], vTc[jt // 8][:, 128 * (jt % 8) : 128 * (jt % 8 + 1)], ident[:, :]
                )
                nc.vector.tensor_copy(out=vp[:, jt, 0, 0:64], in_=tp[:, 0:64])
                nc.vector.tensor_copy(out=vp[:, jt, 1, 0:64], in_=tp[:, 64:128])

            # consumption follows DMA arrival (x second halves land first);
            # att(0) needs kTt, vp[0:16] and qT0 - qT0 is emitted last of those
            proj(kTt[:, 0:1024], 128, HALF)
            spin_mms(4)
            proj(kTt[:, 1024:2048], 128, HALF + 1024)
            spin_mms(4)
            proj(vTc[2][:, :], 256, 2048)
            spin_mms(4)
            proj(vTc[3][:, :], 256, 3072)
            for jt in range(16, 32):
                vtrans(jt)
            proj(vTc[0][:, :], 256, 0)
            proj(vTc[1][:, :], 256, 1024)
            for jt in range(0, 16):
                vtrans(jt)
            for cc in range(4):
                proj(qTc[cc][:, :], 0, 1024 * cc)

            # ---------------- attention + per-chunk output projection --------
            # Emission order software-pipelines the PE: out-proj of chunk cc
            # is emitted AFTER attention of chunk cc+1, so the in-order PE
            # never waits on the DVE normalization chain that produces AT.
            def att_jts(cc):
                voff = 0 if cc < 2 else 16  # spatial: v[0:2048]; temporal: v[2048:]
                avs = [pa.tile([128, 1024], F32, tag="av", name=f"av{h}") for h in range(2)]
                for jt in range(16):
                    ets = []
                    for h in range(2):
                        hp = 64 * h
                        dp = pm.tile([128, 1024], F32, tag="mm", name="dp")
                        for hf in range(2):
                            nc.tensor.matmul(
                                out=dp[:, 512 * hf : 512 * (hf + 1)],
                                lhsT=kTt[hp : hp + 64, 128 * jt : 128 * (jt + 1)],
                                rhs=qTc[cc][hp : hp + 64, 512 * hf : 512 * (hf + 1)],
                                start=True,
                                stop=True,
                            )
                        et = es.tile([128, 1024], BF16, tag="es", name="et")
                        nc.scalar.activation(
                            out=et[:, :], in_=dp[:, :], func=AF.Exp, scale=SCALE
                        )
                        ets.append(et)
                    for h in range(2):
                        for hf in range(2):
                            nc.tensor.matmul(
                                out=avs[h][0:65, 512 * hf : 512 * (hf + 1)],
                                lhsT=vp[:, voff + jt, h, :],
                                rhs=ets[h][:, 512 * hf : 512 * (hf + 1)],
                                start=(jt == 0),
                                stop=(jt == 15),
                            )
                # Stage av to SBUF right away so the PSUM slots free for the
                # next chunk's accumulation; the normalization chain runs off
                # the PE critical path.
                stages = []
                for h in range(2):
                    st = sm.tile([65, 1024], F32, tag="st", name="st")
                    if h == 0:
                        nc.vector.tensor_copy(out=st[:, :], in_=avs[h][0:65, :])
                    else:
                        nc.scalar.copy(out=st[:, :], in_=avs[h][0:65, :])
                    stages.append(st)
                return stages

            # normalize: A^T = av[0:64] / av[64]  (A^T rows 64h..64h+64).
            # The sums row is scatter-DMA'd across 128 partitions so the
            # reciprocal runs 128-lanes-wide (~0.2us, vs 6.5us single-lane,
            # which used to clog the in-order DVE queue and stall out-proj).
            def norm(cc, stages):
                for h in range(2):
                    sd = dr.tile([1024], F32, tag="sd", name="sd")
                    nc.sync.dma_start(out=sd[:], in_=stages[h][64:65, :])
                    spm = sm.tile([128, 8], F32, tag="spm", name="spm")
                    nc.sync.dma_start(
                        out=spm[:, :], in_=sd[:].rearrange("(p t) -> p t", p=128)
                    )
                    rpm = sm.tile([128, 8], F32, tag="rpm", name="rpm")
                    nc.vector.reciprocal(out=rpm[:, :], in_=spm[:, :])
                    rd = dr.tile([1024], F32, tag="rd", name="rd")
                    nc.sync.dma_start(
                        out=rd[:].rearrange("(p t) -> p t", p=128), in_=rpm[:, :]
                    )
                    rb = sm.tile([64, 1024], F32, tag="rb", name="rb")
                    rd_ap = rd[:]
                    rd_b = bass.AP(tensor=rd_ap.tensor, offset=rd_ap.offset,
                                   ap=[[0, 64], [1, 1024]])
                    nc.sync.dma_start(out=rb[:, :], in_=rd_b)
                    nc.vector.tensor_mul(
                        out=ATc[cc][64 * h : 64 * h + 64, :],
                        in0=stages[h][0:64, :],
                        in1=rb[:, :],
                    )
            # output projection for one chunk: P^T[e, n] = Wout_c^T @ A^T
            def outproj(cc):
                for et_ in range(4):
                    for hf in range(2):
                        pool2, tag2 = (pm, "mm") if (et_ * 2 + hf) % 2 == 0 else (pa, "av")
                        ps2 = pool2.tile([128, 512], F32, tag=tag2, name="ps2")
                        nc.tensor.matmul(
                            out=ps2[:, :],
                            lhsT=wout_s[:, 128 * et_ : 128 * (et_ + 1)],
                            rhs=ATc[cc][:, 512 * hf : 512 * (hf + 1)],
                            start=True,
                            stop=True,
                        )
                        ot = osb.tile([128, 512], F32, tag="os", name="ot")
                        # alternate copy engine: ScalarE is idle in the tail
                        if (et_ * 2 + hf) % 2 == 0:
                            nc.vector.tensor_copy(out=ot[:, :], in_=ps2[:, :])
                        else:
                            nc.scalar.copy(out=ot[:, :], in_=ps2[:, :])
                        nc.sync.dma_start(
                            out=outT_d[
                                128 * et_ : 128 * (et_ + 1),
                                1024 * cc + 512 * hf : 1024 * cc + 512 * (hf + 1),
                            ],
                            in_=ot[:, :],
                        )

            # The PE instruction stream through the attention region is pure
            # back-to-back matmuls (norm is DVE/DMA-only and overlaps the next
            # chunk), so the PE never idles long enough for HAM to re-throttle
            # its clock. All output projections run as a short tail; norm(3)
            # is emitted first so its chain overlaps outproj(0..2)'s matmuls.
            st0 = att_jts(0)
            norm(0, st0)
            st1 = att_jts(1)
            norm(1, st1)
            st2 = att_jts(2)
            norm(2, st2)
            st3 = att_jts(3)
            norm(3, st3)
            outproj(0)
            outproj(1)
            outproj(2)
            # keep the PE busy (and the HAM clock warm) while norm(3)'s
            # DMA/DVE chain finishes producing AT3
            spin_mms(10)
            outproj(3)

    nc.compile()
    return nc


_NC = None


def _get_nc():
    global _NC
    if _NC is None:
        _NC = build_nc()
    return _NC


def shard_inputs(x, Wqkv, Wout):
    bf = ml_dtypes.bfloat16
    ins = []
    for core in range(8):
        b, cp = core // 4, core % 4
        hA = 2 * cp
        xT = np.ascontiguousarray(np.asarray(x[b], np.float32).T).astype(bf)
        wq = Wqkv[:, 64 * hA : 64 * hA + 128]
        wk = Wqkv[:, 512 + 64 * hA : 512 + 64 * hA + 128]
        wv = Wqkv[:, 1024 + 64 * hA : 1024 + 64 * hA + 128]
        wqkv_c = np.concatenate([wq, wk, wv], axis=1).astype(bf)
        wout_c = np.ascontiguousarray(Wout[128 * cp : 128 * cp + 128, :], np.float32)
        ins.append({"xT": xT, "Wqkv": wqkv_c, "Wout": wout_c})
    return ins


def run(x, Wqkv, Wout, b_out, trace=False):
    x = np.asarray(x, np.float32)
    Wqkv = np.asarray(Wqkv, np.float32)
    Wout = np.asarray(Wout, np.float32)
    b_out = np.asarray(b_out, np.float32)

    nc = _get_nc()
    ins = shard_inputs(x, Wqkv, Wout)
    res = run_bass_kernel_spmd(nc, ins, list(range(8)), trace=trace)

    out = np.zeros((2, N, DIM), np.float32)
    for core in range(8):
        b = core // 4
        out[b] += res.results[core]["outT"].T
    out += b_out
    return out, res


def kernel(x, Wqkv, Wout, b_out):
    out, _ = run(x, Wqkv, Wout, b_out, trace=False)
    return out

